# revision 11
# baseline (speedup 1.0000x reference)
"""Bipartite GNN attention kernel for Trainium2, SPMD across 8 NeuronCores.

Math (per reference):
  u = user @ W_u.T + b_u ; v = item @ W_v.T + b_v
  learn_user = softmax((u @ v.T) * UV_adj * scale, axis=1) @ v + u
  learn_item = softmax((v @ u.T) * VU_adj * scale, axis=1) @ u + v

Sharding: core i owns rows [i*1024, (i+1)*1024) of BOTH outputs; no
collectives (the contracted-side projection is replicated).

v5 design (fp8 DoubleRow, pipelined):
- All big matmuls (scores, aggregation, denominator, projections) run in
  fp8e4 with perf_mode=DoubleRow (2 k-chunks per instruction).
- Feature matrices are projected twice: fT [h, N] (feature-major, biased,
  score lhsT) and vrow [N, h] (row-major, UNbiased, aggregation rhs).
  The missing bias in vrow cancels through softmax:
  P@(v + 1 b^T)/rsum = P@vrow/rsum + b^T, so b_feat is folded into the
  residual qrow instead. No per-block PE transposes anywhere.
- Per-core inputs are column-ROLLED so this core's rows are columns
  [0:RB) of both feature matrices; the score rhs (qTb) is then just
  fT_other[:, :, 0:RB] - no separate query projection.
- exp uses bias -ln(32): softmax is shift-invariant, masked entries
  become exactly 1/32 (fp8-exact), max value ~5 stays far below fp8e4
  max 240.
- Attention works on 256-user row blocks: a score pair (2 item chunks)
  fits one PSUM bank, so mask-mult and exp are ONE DVE op + ONE Act op
  per pair. The denominator is ones^T @ P (ones stationary -> 2-column
  weight load), giving a [1,256] row transposed back via two 1-partition
  matmuls in the epilogue.
- Aggregation of pair bp-3 is emitted after scores of pair bp: the
  DVE-mult + Act-exp latency (~1.4us) spans 3 pipeline periods, so the
  PE never stalls on it.
- PSUM pools are phase-scoped: projections use a 3-deep ring of 2-bank
  tiles (paired casts amortize DVE/Act per-op overhead), attention uses
  agg[2] + scores[3] + rsum[1] + aux[1].
"""

import sys

sys.path.insert(0, "/opt/trn_rl_repo")

import ml_dtypes
import numpy as np

import concourse.bacc as bacc
import concourse.bass as bass
import concourse.mybir as mybir
import concourse.tile as tile
from concourse.bass_utils import run_bass_kernel_spmd

N = 8192          # users == items
H = 512           # hidden
NCORES = 8
RB = N // NCORES  # 1024 rows per core per direction
KH = H // 128     # 4 h-chunks
NB = N // 128     # 64 column chunks
NBP = NB // 2     # 32 column-pair chunks (DoubleRow)
RBQ = 256         # users per attention row-block
NRB = RB // RBQ   # 4 r-blocks of 256
NJ = N // 512     # 16 512-col blocks for projection streaming
AGG_DEPTH = 3     # aggregation trails scores by this many pairs
SCALE = float(1.0 / np.sqrt(np.float32(H)))
NLN32 = float(-np.log(32.0))

F32 = mybir.dt.float32
F32R = mybir.dt.float32r
BF16 = mybir.dt.bfloat16
FP8 = mybir.dt.float8e4
NP_FP8 = ml_dtypes.float8_e4m3
DR = mybir.MatmulPerfMode.DoubleRow


def _r(ap):
    return ap.bitcast(F32R)


def build_nc():
    nc = bacc.Bacc("TRN2", target_bir_lowering=False, debug=False)

    featA = nc.declare_dram_parameter("featA", [H, N], FP8, isOutput=False)
    featB = nc.declare_dram_parameter("featB", [H, N], FP8, isOutput=False)
    qtA = nc.declare_dram_parameter("qtA", [H, RB], F32, isOutput=False)
    qtB = nc.declare_dram_parameter("qtB", [H, RB], F32, isOutput=False)
    maskA = nc.declare_dram_parameter("maskA", [N, RB], BF16, isOutput=False)
    maskB = nc.declare_dram_parameter("maskB", [N, RB], BF16, isOutput=False)
    WfA = nc.declare_dram_parameter("WfA", [128, KH, H], FP8, isOutput=False)
    WfB = nc.declare_dram_parameter("WfB", [128, KH, H], FP8, isOutput=False)
    WqA = nc.declare_dram_parameter("WqA", [H, H], F32, isOutput=False)
    WqB = nc.declare_dram_parameter("WqB", [H, H], F32, isOutput=False)
    bfA = nc.declare_dram_parameter("bfA", [128, KH], F32, isOutput=False)
    bfB = nc.declare_dram_parameter("bfB", [128, KH], F32, isOutput=False)
    bfA_rep = nc.declare_dram_parameter("bfA_rep", [128, 2, H], F32,
                                        isOutput=False)
    bfB_rep = nc.declare_dram_parameter("bfB_rep", [128, 2, H], F32,
                                        isOutput=False)
    brow = nc.declare_dram_parameter("brow", [128, 2, H], F32, isOutput=False)
    out = nc.declare_dram_parameter("out", [2 * RB, H], F32, isOutput=True)

    with tile.TileContext(nc) as tc:
        with (
            tc.tile_pool(name="bigA", bufs=1) as bigA,
            tc.tile_pool(name="bigB", bufs=1) as bigB,
            tc.tile_pool(name="wts", bufs=1) as wts,
            tc.tile_pool(name="stream", bufs=4) as stream,
            tc.tile_pool(name="qstream", bufs=2) as qstream,
            tc.tile_pool(name="mask", bufs=4) as maskp,
            tc.tile_pool(name="pf", bufs=4) as pfp,
            tc.tile_pool(name="pb", bufs=5) as pbp,
            tc.tile_pool(name="outs", bufs=1) as outsp,
            tc.tile_pool(name="small", bufs=1) as small,
        ):
            ones2 = small.tile([128, 2, 16], FP8, tag="ones")
            nc.vector.memset(ones2[:], 1.0)
            onesf = small.tile([1, 1], F32, tag="onesf")
            nc.vector.memset(onesf[:], 1.0)
            nbias = small.tile([128, 1], F32, tag="nbias")
            nc.vector.memset(nbias[:], NLN32)
            brow_sb = small.tile([128, 2, H], F32, tag="brow")
            nc.sync.dma_start(brow_sb[:], brow[:])
            bf_sb = {}
            bfrep_sb = {}
            for d, nar, rep in (("A", bfA, bfA_rep), ("B", bfB, bfB_rep)):
                bf_sb[d] = small.tile([128, KH], F32, tag=f"bf{d}",
                                      name=f"bf{d}sb")
                nc.sync.dma_start(bf_sb[d][:], nar[:])
                bfrep_sb[d] = small.tile([128, 2, H], F32, tag=f"bfr{d}",
                                         name=f"bfr{d}sb")
                nc.sync.dma_start(bfrep_sb[d][:], rep[:])

            # persistent per-direction tensors
            fT = {}
            vrow = {}
            qrow = {}
            for big_pool, d in ((bigA, "A"), (bigB, "B")):
                fT[d] = big_pool.tile([128, KH, N], FP8, tag=f"fT{d}",
                                      name=f"fT{d}")
                vrow[d] = big_pool.tile([128, NB, H], FP8, tag=f"vrow{d}",
                                        name=f"vrow{d}")
                qrow[d] = big_pool.tile([128, 2 * KH, H], BF16, tag=f"qrow{d}",
                                        name=f"qrow{d}")

            # ---------------- phase 0: projections ----------------
            def project(ps_w, d, feat_dram, qt_dram, wf_dram, wq_dram):
                wfp = wts.tile([128, KH, H], FP8, tag="wfp", name=f"wfp{d}")
                nc.sync.dma_start(wfp[:], wf_dram[:])
                wq = [wts.tile([128, H], F32R, tag=f"wq{k}", name=f"wq{d}{k}")
                      for k in range(KH)]
                for k in range(KH):
                    nc.sync.dma_start(
                        wq[k][:], wq_dram[k * 128:(k + 1) * 128, :].bitcast(F32R))

                # qrow: residual projection, f32r for accuracy; bias row
                # (b_q + b_feat) added from a replicated tile
                for half in range(2):
                    qt_in = [qstream.tile([128, 512], F32R, tag=f"qt{k}",
                                          name=f"qt{d}{half}{k}")
                             for k in range(KH)]
                    for k in range(KH):
                        nc.sync.dma_start(
                            qt_in[k][:],
                            qt_dram[k * 128:(k + 1) * 128,
                                    half * 512:(half + 1) * 512].bitcast(F32R))
                    for cp in range(2):  # pairs of 128-row chunks
                        ps = ps_w.tile([128, 2, H], F32, tag="w")
                        for i in range(2):
                            c = cp * 2 + i
                            for k in range(KH):
                                nc.tensor.matmul(
                                    ps[:, i, :],
                                    qt_in[k][:, c * 128:(c + 1) * 128],
                                    wq[k][:],
                                    start=(k == 0), stop=(k == KH - 1))
                        nc.vector.tensor_tensor(
                            out=qrow[d][:, half * 4 + cp * 2:
                                        half * 4 + cp * 2 + 2, :],
                            in0=ps[:], in1=brow_sb[:],
                            op=mybir.AluOpType.add)

                # fT + vrow: fp8 DoubleRow projections with paired casts
                for j in range(NJ):
                    ft_in = stream.tile([128, KH, 512], FP8, tag="ft",
                                        name=f"ft{d}{j}")
                    for k in range(KH):
                        nc.sync.dma_start(
                            ft_in[:, k, :],
                            feat_dram[k * 128:(k + 1) * 128,
                                      j * 512:(j + 1) * 512])
                    for mp in range(2):  # m-pairs
                        ps = ps_w.tile([128, 2, 512], F32, tag="w")
                        for i in range(2):
                            m = 2 * mp + i
                            for ko in range(2):
                                nc.tensor.matmul(
                                    ps[:, i, :],
                                    wfp[:, 2 * ko:2 * ko + 2,
                                        m * 128:(m + 1) * 128],
                                    ft_in[:, 2 * ko:2 * ko + 2, :],
                                    start=(ko == 0), stop=(ko == 1),
                                    perf_mode=DR)
                        if mp == 0:
                            # DVE: wide cast with replicated bias
                            nc.vector.tensor_tensor(
                                out=fT[d][:, 0:2, j * 512:(j + 1) * 512],
                                in0=ps[:], in1=bfrep_sb[d][:],
                                op=mybir.AluOpType.add)
                        else:
                            # Act: two narrow casts with per-partition bias
                            for i in range(2):
                                m = 2 * mp + i
                                nc.scalar.add(
                                    fT[d][:, m, j * 512:(j + 1) * 512],
                                    ps[:, i, :], bf_sb[d][:, m:m + 1])
                    for sp in range(2):  # pairs of 128-item chunks
                        ps = ps_w.tile([128, 2, 512], F32, tag="w")
                        for i in range(2):
                            sub = sp * 2 + i
                            for ko in range(2):
                                nc.tensor.matmul(
                                    ps[:, i, :],
                                    ft_in[:, 2 * ko:2 * ko + 2,
                                          sub * 128:(sub + 1) * 128],
                                    wfp[:, 2 * ko:2 * ko + 2, :],
                                    start=(ko == 0), stop=(ko == 1),
                                    perf_mode=DR)
                        c = j * 4 + sp * 2
                        if sp == 0:
                            nc.vector.tensor_copy(vrow[d][:, c:c + 2, :], ps[:])
                        else:
                            nc.scalar.copy(vrow[d][:, c:c + 2, :], ps[:])

            with tc.tile_pool(name="ps_w", bufs=3, space="PSUM") as ps_w:
                project(ps_w, "A", featA, qtA, WfA, WqA)
                project(ps_w, "B", featB, qtB, WfB, WqB)

            # ---------------- attention ----------------
            def emit_agg(myvrow, agg, rs_row, pbf2, bp):
                for rs in range(2):
                    nc.tensor.matmul(
                        agg[:, rs, :], pbf2[:, :, rs * 128:(rs + 1) * 128],
                        myvrow[:, 2 * bp:2 * bp + 2, :],
                        start=(bp == 0), stop=(bp == NBP - 1), perf_mode=DR)
                # denominator: ones^T @ pbf2 -> [1, RBQ] row; ones is the
                # stationary operand so the weight load is only 2 columns
                nc.tensor.matmul(
                    rs_row[:], ones2[:, :, 0:1], pbf2[:],
                    start=(bp == 0), stop=(bp == NBP - 1), perf_mode=DR)

            def attention(ps_s, ps_agg, ps_rs, ps_aux, d, other, mask_dram,
                          out_base):
                myfT = fT[d]
                myvrow = vrow[d]
                qTb = fT[other]
                for rb in range(NRB):
                    agg = ps_agg.tile([128, 2, 512], F32, tag="agg")
                    rs_row = ps_rs.tile([1, RBQ], F32, tag="rs")
                    pend = []  # pbf2 pairs waiting for aggregation
                    for bp in range(NBP):
                        # both score halves share one psum bank = ONE
                        # accumulation group: start on first, stop on last
                        sps = ps_s.tile([128, 2, RBQ], F32, tag="s")
                        for t in range(2):
                            b = 2 * bp + t
                            for ko in range(2):
                                nc.tensor.matmul(
                                    sps[:, t, :],
                                    myfT[:, 2 * ko:2 * ko + 2,
                                         b * 128:(b + 1) * 128],
                                    qTb[:, 2 * ko:2 * ko + 2,
                                        rb * RBQ:(rb + 1) * RBQ],
                                    start=(t == 0 and ko == 0),
                                    stop=(t == 1 and ko == 1),
                                    perf_mode=DR)

                        # aggregate an OLD pair: its exp had AGG_DEPTH
                        # periods to finish, so the PE never waits
                        if len(pend) >= AGG_DEPTH:
                            emit_agg(myvrow, agg, rs_row, *pend.pop(0))
                        mt = maskp.tile([128, 2, RBQ], BF16, tag="mk")
                        for t in range(2):
                            nc.sync.dma_start(
                                mt[:, t, :],
                                mask_dram[(2 * bp + t) * 128:
                                          (2 * bp + t + 1) * 128,
                                          rb * RBQ:(rb + 1) * RBQ])
                        pbf2 = pbp.tile([128, 2, RBQ], FP8, tag="pbf")
                        p16 = pfp.tile([128, 2, RBQ], BF16, tag="p16")
                        nc.vector.tensor_tensor(
                            out=p16[:], in0=sps[:], in1=mt[:],
                            op=mybir.AluOpType.mult)
                        nc.scalar.activation(
                            pbf2[:], p16[:],
                            mybir.ActivationFunctionType.Exp,
                            bias=nbias[:], scale=SCALE)
                        pend.append((pbf2, bp))
                    for p in pend:
                        emit_agg(myvrow, agg, rs_row, *p)

                    # epilogue: transpose [1, RBQ] denominators to [128, 2]
                    # via two 1-partition matmuls, then out = agg/rsum + qrow
                    rs_sb = small.tile([1, RBQ], F32, tag="rs_sb")
                    nc.vector.tensor_copy(rs_sb[:], rs_row[:])
                    rsT = ps_aux.tile([128, 2], F32, tag="rsT")
                    for rs in range(2):
                        nc.tensor.matmul(
                            rsT[:, rs:rs + 1],
                            rs_sb[0:1, rs * 128:(rs + 1) * 128], onesf[:],
                            start=(rs == 0), stop=(rs == 1))
                    recip = small.tile([128, 2], F32, tag="recip")
                    nc.vector.reciprocal(recip[:], rsT[:])
                    for rs in range(2):
                        o_sb = outsp.tile([128, H], F32, tag=f"o{rs}",
                                          name=f"o{d}{rb}_{rs}")
                        if rs % 2 == 0:
                            nc.vector.tensor_scalar(
                                out=o_sb[:], in0=agg[:, rs, :],
                                scalar1=recip[:, rs:rs + 1], scalar2=None,
                                op0=mybir.AluOpType.mult)
                            nc.vector.tensor_tensor(
                                out=o_sb[:], in0=o_sb[:],
                                in1=qrow[d][:, rb * 2 + rs, :],
                                op=mybir.AluOpType.add)
                        else:
                            nc.scalar.mul(o_sb[:], agg[:, rs, :],
                                          recip[:, rs:rs + 1])
                            nc.gpsimd.tensor_tensor(
                                out=o_sb[:], in0=o_sb[:],
                                in1=qrow[d][:, rb * 2 + rs, :],
                                op=mybir.AluOpType.add)
                        row0 = out_base + rb * RBQ + rs * 128
                        nc.sync.dma_start(out[row0:row0 + 128, :], o_sb[:])

            with (
                tc.tile_pool(name="ps_s", bufs=3, space="PSUM") as ps_s,
                tc.tile_pool(name="ps_agg", bufs=1, space="PSUM") as ps_agg,
                tc.tile_pool(name="ps_rs", bufs=1, space="PSUM") as ps_rs,
                tc.tile_pool(name="ps_aux", bufs=1, space="PSUM") as ps_aux,
            ):
                attention(ps_s, ps_agg, ps_rs, ps_aux, "A", "B", maskA, 0)
                attention(ps_s, ps_agg, ps_rs, ps_aux, "B", "A", maskB, RB)

    nc.compile()
    return nc


_NC_CACHE = None
TRACE = False
LAST_RESULT = None


def kernel(user, item, UV_adj, VU_adj, W_u, b_u, W_v, b_v):
    global _NC_CACHE, LAST_RESULT
    user = np.asarray(user, dtype=np.float32)
    item = np.asarray(item, dtype=np.float32)
    UV_adj = np.asarray(UV_adj, dtype=np.float32)
    VU_adj = np.asarray(VU_adj, dtype=np.float32)
    W_u = np.asarray(W_u, dtype=np.float32)
    W_v = np.asarray(W_v, dtype=np.float32)
    b_u = np.asarray(b_u, dtype=np.float32)
    b_v = np.asarray(b_v, dtype=np.float32)

    userT = np.ascontiguousarray(user.T)
    itemT = np.ascontiguousarray(item.T)
    userT8 = userT.astype(NP_FP8)
    itemT8 = itemT.astype(NP_FP8)
    UV16 = UV_adj.astype(ml_dtypes.bfloat16)
    VU16 = np.ascontiguousarray(UV16.T)
    W_uT = np.ascontiguousarray(W_u.T)
    W_vT = np.ascontiguousarray(W_v.T)
    # [128, KH, H] fp8 weight layout for DoubleRow projections
    WfA_np = np.ascontiguousarray(
        W_vT.reshape(KH, 128, H).transpose(1, 0, 2).astype(NP_FP8))
    WfB_np = np.ascontiguousarray(
        W_uT.reshape(KH, 128, H).transpose(1, 0, 2).astype(NP_FP8))
    bfA_np = np.ascontiguousarray(b_v.reshape(KH, 128).T)
    bfB_np = np.ascontiguousarray(b_u.reshape(KH, 128).T)
    # bias for m-chunks 0,1 replicated along the free dim (DVE wide casts)
    bfA_rep_np = np.ascontiguousarray(
        np.broadcast_to(b_v.reshape(KH, 128).T[:, 0:2, None], (128, 2, H)))
    bfB_rep_np = np.ascontiguousarray(
        np.broadcast_to(b_u.reshape(KH, 128).T[:, 0:2, None], (128, 2, H)))
    brow_np = np.ascontiguousarray(
        np.broadcast_to((b_u + b_v)[None, None, :], (128, 2, H)))

    in_maps = []
    for i in range(NCORES):
        r = i * RB
        sl = slice(r, r + RB)
        in_maps.append({
            # feature matrices with this core's rows rolled to the front
            "featA": np.ascontiguousarray(np.roll(itemT8, -r, axis=1)),
            "featB": np.ascontiguousarray(np.roll(userT8, -r, axis=1)),
            "qtA": np.ascontiguousarray(userT[:, sl]),
            "qtB": np.ascontiguousarray(itemT[:, sl]),
            "maskA": np.ascontiguousarray(np.roll(VU16[:, sl], -r, axis=0)),
            "maskB": np.ascontiguousarray(np.roll(UV16[:, sl], -r, axis=0)),
            "WfA": WfA_np,
            "WfB": WfB_np,
            "WqA": W_uT,
            "WqB": W_vT,
            "bfA": bfA_np,
            "bfB": bfB_np,
            "bfA_rep": bfA_rep_np,
            "bfB_rep": bfB_rep_np,
            "brow": brow_np,
        })

    if _NC_CACHE is None:
        _NC_CACHE = build_nc()
    res = run_bass_kernel_spmd(_NC_CACHE, in_maps, core_ids=list(range(NCORES)),
                               trace=TRACE)
    LAST_RESULT = res
    results = res.results
    learn_user = np.concatenate([results[i]["out"][:RB] for i in range(NCORES)], 0)
    learn_item = np.concatenate([results[i]["out"][RB:] for i in range(NCORES)], 0)
    return (learn_user, learn_item)


if __name__ == "__main__":
    nc = build_nc()
    print("built ok")


# revision 14
# speedup vs baseline: 1.2023x; 1.2023x over previous
"""Bipartite GNN attention kernel for Trainium2, SPMD across 8 NeuronCores.

Math (per reference):
  u = user @ W_u.T + b_u ; v = item @ W_v.T + b_v
  learn_user = softmax((u @ v.T) * UV_adj * scale, axis=1) @ v + u
  learn_item = softmax((v @ u.T) * VU_adj * scale, axis=1) @ u + v

Sharding: core i owns rows [i*1024, (i+1)*1024) of BOTH outputs; no
collectives (the contracted-side projection is replicated).

v6 design (fp8 DoubleRow, long streams):
- All big matmuls run in fp8e4 DoubleRow with 512-column outputs: long
  moving streams hide the (FWL-less) DoubleRow weight loads. 256-column
  variants measured issue/LDW-bound, not stream-bound.
- Feature matrices are projected twice: fT [h, N] (feature-major, biased,
  score lhsT) and vrow [N, h] (row-major, UNbiased, aggregation rhs).
  The missing bias in vrow cancels through softmax:
  P@(v + 1 b^T)/rsum = P@vrow/rsum + b^T, so b_feat is folded into the
  residual qrow instead. No per-block PE transposes anywhere.
- Per-core inputs are column-ROLLED so this core's rows are columns
  [0:RB) of both feature matrices; the score rhs (qTb) is then just
  fT_other[:, :, 0:RB] - no separate query projection.
- exp uses bias -ln(32): softmax is shift-invariant, masked entries
  become exactly 1/32 (fp8-exact), max value ~5 stays far below fp8e4
  max 240. Both exp halves of an item-chunk pair run as ONE Act op into
  a [128, 2, 512] fp8 pair tile (SBUF, so pairing is free).
- Denominator: ones^T @ pbf_pair, ONE DoubleRow matmul per pair with the
  2-column ones as the stationary side, accumulating a [1, 512] PSUM row;
  the epilogue transposes it to [128, 4] with four 1-partition matmuls.
- Aggregation of pair bp-2 is emitted after scores of pair bp so the PE
  never waits on the DVE-mult + Act-exp chain.
- All DMAs are fully contiguous: host pre-tiles masks per row-block and
  features per 512-column block.
- PSUM pools are phase-scoped: projections use a 3-deep ring of 2-bank
  tiles (paired casts amortize DVE/Act per-op overhead); attention uses
  agg[4] + scores[3] + rsum-row[1], with the epilogue transpose borrowing
  a scores slot.
"""

import sys

sys.path.insert(0, "/opt/trn_rl_repo")

import ml_dtypes
import numpy as np

import concourse.bacc as bacc
import concourse.bass as bass
import concourse.mybir as mybir
import concourse.tile as tile
from concourse.bass_utils import run_bass_kernel_spmd

N = 8192          # users == items
H = 512           # hidden
NCORES = 8
RB = N // NCORES  # 1024 rows per core per direction
KH = H // 128     # 4 h-chunks
NB = N // 128     # 64 column chunks
NBP = NB // 2     # 32 column-pair chunks (DoubleRow)
RBQ = 512         # users per attention row-block
NRB = RB // RBQ   # 2 r-blocks of 512
NJ = N // 512     # 16 512-col blocks for projection streaming
AGG_DEPTH = 2     # aggregation trails scores by this many pairs
SCALE = float(1.0 / np.sqrt(np.float32(H)))
NLN32 = float(-np.log(32.0))

F32 = mybir.dt.float32
F32R = mybir.dt.float32r
BF16 = mybir.dt.bfloat16
FP8 = mybir.dt.float8e4
NP_FP8 = ml_dtypes.float8_e4m3
DR = mybir.MatmulPerfMode.DoubleRow


def _r(ap):
    return ap.bitcast(F32R)


def build_nc():
    nc = bacc.Bacc("TRN2", target_bir_lowering=False, debug=False)

    featA = nc.declare_dram_parameter("featA", [NJ, H, 512], FP8, isOutput=False)
    featB = nc.declare_dram_parameter("featB", [NJ, H, 512], FP8, isOutput=False)
    qtA = nc.declare_dram_parameter("qtA", [2, H, 512], F32, isOutput=False)
    qtB = nc.declare_dram_parameter("qtB", [2, H, 512], F32, isOutput=False)
    maskA = nc.declare_dram_parameter("maskA", [NRB, N, RBQ], FP8, isOutput=False)
    maskB = nc.declare_dram_parameter("maskB", [NRB, N, RBQ], FP8, isOutput=False)
    WfA = nc.declare_dram_parameter("WfA", [128, KH, H], FP8, isOutput=False)
    WfB = nc.declare_dram_parameter("WfB", [128, KH, H], FP8, isOutput=False)
    WqA = nc.declare_dram_parameter("WqA", [H, H], F32, isOutput=False)
    WqB = nc.declare_dram_parameter("WqB", [H, H], F32, isOutput=False)
    bfA = nc.declare_dram_parameter("bfA", [128, KH], F32, isOutput=False)
    bfB = nc.declare_dram_parameter("bfB", [128, KH], F32, isOutput=False)
    bfA_rep = nc.declare_dram_parameter("bfA_rep", [128, 2, H], F32,
                                        isOutput=False)
    bfB_rep = nc.declare_dram_parameter("bfB_rep", [128, 2, H], F32,
                                        isOutput=False)
    brow = nc.declare_dram_parameter("brow", [128, 2, H], F32, isOutput=False)
    out = nc.declare_dram_parameter("out", [2 * RB, H], F32, isOutput=True)

    with tile.TileContext(nc) as tc:
        with (
            tc.tile_pool(name="bigA", bufs=1) as bigA,
            tc.tile_pool(name="bigB", bufs=1) as bigB,
            tc.tile_pool(name="wts", bufs=1) as wts,
            tc.tile_pool(name="stream", bufs=4) as stream,
            tc.tile_pool(name="qstream", bufs=1) as qstream,
            tc.tile_pool(name="mask", bufs=3) as maskp,
            tc.tile_pool(name="pf", bufs=3) as pfp,
            tc.tile_pool(name="pb", bufs=4) as pbp,
            tc.tile_pool(name="outs", bufs=1) as outsp,
            tc.tile_pool(name="small", bufs=1) as small,
        ):
            ones2 = small.tile([128, 2, 16], FP8, tag="ones")
            nc.vector.memset(ones2[:], 1.0)
            onesf = small.tile([1, 1], F32, tag="onesf")
            nc.vector.memset(onesf[:], 1.0)
            nbias = small.tile([128, 1], F32, tag="nbias")
            nc.vector.memset(nbias[:], NLN32)
            brow_sb = small.tile([128, 2, H], F32, tag="brow")
            nc.sync.dma_start(brow_sb[:], brow[:])
            bf_sb = {}
            bfrep_sb = {}
            for d, nar, rep in (("A", bfA, bfA_rep), ("B", bfB, bfB_rep)):
                bf_sb[d] = small.tile([128, KH], F32, tag=f"bf{d}",
                                      name=f"bf{d}sb")
                nc.sync.dma_start(bf_sb[d][:], nar[:])
                bfrep_sb[d] = small.tile([128, 2, H], F32, tag=f"bfr{d}",
                                         name=f"bfr{d}sb")
                nc.sync.dma_start(bfrep_sb[d][:], rep[:])

            # persistent per-direction tensors
            fT = {}
            vrow = {}
            qrow = {}
            for big_pool, d in ((bigA, "A"), (bigB, "B")):
                fT[d] = big_pool.tile([128, KH, N], FP8, tag=f"fT{d}",
                                      name=f"fT{d}")
                vrow[d] = big_pool.tile([128, NB, H], FP8, tag=f"vrow{d}",
                                        name=f"vrow{d}")
                qrow[d] = big_pool.tile([128, 2 * KH, H], BF16, tag=f"qrow{d}",
                                        name=f"qrow{d}")

            # ---------------- phase 0: projections ----------------
            def project(ps_w, d, feat_dram, qt_dram, wf_dram, wq_dram):
                wfp = wts.tile([128, KH, H], FP8, tag="wfp", name=f"wfp{d}")
                nc.sync.dma_start(wfp[:], wf_dram[:])
                wq = [wts.tile([128, H], F32R, tag=f"wq{k}", name=f"wq{d}{k}")
                      for k in range(KH)]
                for k in range(KH):
                    nc.sync.dma_start(
                        wq[k][:], wq_dram[k * 128:(k + 1) * 128, :].bitcast(F32R))

                # qrow: residual projection, f32r for accuracy; bias row
                # (b_q + b_feat) added from a replicated tile
                for half in range(2):
                    qt_in = [qstream.tile([128, 512], F32R, tag=f"qt{k}",
                                          name=f"qt{d}{half}{k}")
                             for k in range(KH)]
                    for k in range(KH):
                        nc.sync.dma_start(
                            qt_in[k][:],
                            qt_dram[half, k * 128:(k + 1) * 128, :].bitcast(F32R))
                    for cp in range(2):  # pairs of 128-row chunks
                        ps = ps_w.tile([128, 2, H], F32, tag="w")
                        for i in range(2):
                            c = cp * 2 + i
                            for k in range(KH):
                                nc.tensor.matmul(
                                    ps[:, i, :],
                                    qt_in[k][:, c * 128:(c + 1) * 128],
                                    wq[k][:],
                                    start=(k == 0), stop=(k == KH - 1))
                        nc.vector.tensor_tensor(
                            out=qrow[d][:, half * 4 + cp * 2:
                                        half * 4 + cp * 2 + 2, :],
                            in0=ps[:], in1=brow_sb[:],
                            op=mybir.AluOpType.add)

                # fT + vrow: fp8 DoubleRow projections with paired casts
                for j in range(NJ):
                    ft_in = stream.tile([128, KH, 512], FP8, tag="ft",
                                        name=f"ft{d}{j}")
                    for k in range(KH):
                        nc.sync.dma_start(
                            ft_in[:, k, :],
                            feat_dram[j, k * 128:(k + 1) * 128, :])
                    for mp in range(2):  # m-pairs
                        ps = ps_w.tile([128, 2, 512], F32, tag="w")
                        for i in range(2):
                            m = 2 * mp + i
                            for ko in range(2):
                                nc.tensor.matmul(
                                    ps[:, i, :],
                                    wfp[:, 2 * ko:2 * ko + 2,
                                        m * 128:(m + 1) * 128],
                                    ft_in[:, 2 * ko:2 * ko + 2, :],
                                    start=(ko == 0), stop=(ko == 1),
                                    perf_mode=DR)
                        if mp == 0:
                            # DVE: wide cast with replicated bias
                            nc.vector.tensor_tensor(
                                out=fT[d][:, 0:2, j * 512:(j + 1) * 512],
                                in0=ps[:], in1=bfrep_sb[d][:],
                                op=mybir.AluOpType.add)
                        else:
                            # Act: two narrow casts with per-partition bias
                            for i in range(2):
                                m = 2 * mp + i
                                nc.scalar.add(
                                    fT[d][:, m, j * 512:(j + 1) * 512],
                                    ps[:, i, :], bf_sb[d][:, m:m + 1])
                    for sp in range(2):  # pairs of 128-item chunks
                        ps = ps_w.tile([128, 2, 512], F32, tag="w")
                        for i in range(2):
                            sub = sp * 2 + i
                            for ko in range(2):
                                nc.tensor.matmul(
                                    ps[:, i, :],
                                    ft_in[:, 2 * ko:2 * ko + 2,
                                          sub * 128:(sub + 1) * 128],
                                    wfp[:, 2 * ko:2 * ko + 2, :],
                                    start=(ko == 0), stop=(ko == 1),
                                    perf_mode=DR)
                        c = j * 4 + sp * 2
                        if sp == 0:
                            nc.vector.tensor_copy(vrow[d][:, c:c + 2, :], ps[:])
                        else:
                            nc.scalar.copy(vrow[d][:, c:c + 2, :], ps[:])

            with tc.tile_pool(name="ps_w", bufs=3, space="PSUM") as ps_w:
                project(ps_w, "A", featA, qtA, WfA, WqA)
                project(ps_w, "B", featB, qtB, WfB, WqB)

            # ---------------- attention ----------------
            def emit_agg(myvrow, agg, rs_row, pbf2, bp):
                for rs in range(4):
                    nc.tensor.matmul(
                        agg[:, rs, :], pbf2[:, :, rs * 128:(rs + 1) * 128],
                        myvrow[:, 2 * bp:2 * bp + 2, :],
                        start=(bp == 0), stop=(bp == NBP - 1), perf_mode=DR)
                # denominator: ones^T @ pbf2 -> [1, RBQ] row; ones is the
                # stationary operand so the weight load is only 2 columns
                nc.tensor.matmul(
                    rs_row[:], ones2[:, :, 0:1], pbf2[:],
                    start=(bp == 0), stop=(bp == NBP - 1), perf_mode=DR)

            def attention(ps_s, ps_agg, ps_rs, d, other, mask_dram, out_base):
                myfT = fT[d]
                myvrow = vrow[d]
                qTb = fT[other]
                for rb in range(NRB):
                    agg = ps_agg.tile([128, KH, 512], F32, tag="agg")
                    rs_row = ps_rs.tile([1, RBQ], F32, tag="rs")
                    pend = []  # pbf pairs waiting for aggregation
                    for bp in range(NBP):
                        sps = []
                        for t in range(2):
                            b = 2 * bp + t
                            sp = ps_s.tile([128, 512], F32, tag="s")
                            for ko in range(2):
                                nc.tensor.matmul(
                                    sp[:],
                                    myfT[:, 2 * ko:2 * ko + 2,
                                         b * 128:(b + 1) * 128],
                                    qTb[:, 2 * ko:2 * ko + 2,
                                        rb * RBQ:(rb + 1) * RBQ],
                                    start=(ko == 0), stop=(ko == 1),
                                    perf_mode=DR)
                            sps.append(sp)

                        # aggregate an OLD pair: its exp had AGG_DEPTH
                        # periods to finish, so the PE never waits
                        if len(pend) >= AGG_DEPTH:
                            emit_agg(myvrow, agg, rs_row, *pend.pop(0))
                        mt = maskp.tile([128, 2, RBQ], FP8, tag="mk")
                        for t in range(2):
                            nc.sync.dma_start(
                                mt[:, t, :],
                                mask_dram[rb, (2 * bp + t) * 128:
                                          (2 * bp + t + 1) * 128, :])
                        pbf2 = pbp.tile([128, 2, RBQ], FP8, tag="pbf")
                        p16 = pfp.tile([128, 2, RBQ], BF16, tag="p16")
                        for t in range(2):
                            nc.vector.tensor_tensor(
                                out=p16[:, t, :], in0=sps[t][:],
                                in1=mt[:, t, :], op=mybir.AluOpType.mult)
                        # ONE exp for both halves (contiguous SBUF pair)
                        nc.scalar.activation(
                            pbf2[:], p16[:],
                            mybir.ActivationFunctionType.Exp,
                            bias=nbias[:], scale=SCALE)
                        pend.append((pbf2, bp))
                    for p in pend:
                        emit_agg(myvrow, agg, rs_row, *p)

                    # epilogue: transpose [1, RBQ] denominators to [128, 4]
                    # via 1-partition matmuls, then out = agg/rsum + qrow
                    rs_sb = small.tile([1, RBQ], F32, tag="rs_sb")
                    nc.vector.tensor_copy(rs_sb[:], rs_row[:])
                    rsT = ps_s.tile([128, 4], F32, tag="s", name=f"rsT{d}{rb}")
                    for rs in range(4):
                        nc.tensor.matmul(
                            rsT[:, rs:rs + 1],
                            rs_sb[0:1, rs * 128:(rs + 1) * 128], onesf[:],
                            start=(rs == 0), stop=(rs == 3))
                    recip = small.tile([128, 4], F32, tag="recip")
                    nc.vector.reciprocal(recip[:], rsT[:])
                    for rs in range(4):
                        o_sb = outsp.tile([128, H], F32, tag=f"o{rs}",
                                          name=f"o{d}{rb}_{rs}")
                        if rs % 2 == 0:
                            nc.vector.tensor_scalar(
                                out=o_sb[:], in0=agg[:, rs, :],
                                scalar1=recip[:, rs:rs + 1], scalar2=None,
                                op0=mybir.AluOpType.mult)
                            nc.vector.tensor_tensor(
                                out=o_sb[:], in0=o_sb[:],
                                in1=qrow[d][:, rb * 4 + rs, :],
                                op=mybir.AluOpType.add)
                        else:
                            nc.scalar.mul(o_sb[:], agg[:, rs, :],
                                          recip[:, rs:rs + 1])
                            nc.gpsimd.tensor_tensor(
                                out=o_sb[:], in0=o_sb[:],
                                in1=qrow[d][:, rb * 4 + rs, :],
                                op=mybir.AluOpType.add)
                        row0 = out_base + rb * RBQ + rs * 128
                        nc.sync.dma_start(out[row0:row0 + 128, :], o_sb[:])

            with (
                tc.tile_pool(name="ps_s", bufs=3, space="PSUM") as ps_s,
                tc.tile_pool(name="ps_agg", bufs=1, space="PSUM") as ps_agg,
                tc.tile_pool(name="ps_rs", bufs=1, space="PSUM") as ps_rs,
            ):
                attention(ps_s, ps_agg, ps_rs, "A", "B", maskA, 0)
                attention(ps_s, ps_agg, ps_rs, "B", "A", maskB, RB)

    nc.compile()
    return nc


_NC_CACHE = None
TRACE = False
LAST_RESULT = None


def kernel(user, item, UV_adj, VU_adj, W_u, b_u, W_v, b_v):
    global _NC_CACHE, LAST_RESULT
    user = np.asarray(user, dtype=np.float32)
    item = np.asarray(item, dtype=np.float32)
    UV_adj = np.asarray(UV_adj, dtype=np.float32)
    VU_adj = np.asarray(VU_adj, dtype=np.float32)
    W_u = np.asarray(W_u, dtype=np.float32)
    W_v = np.asarray(W_v, dtype=np.float32)
    b_u = np.asarray(b_u, dtype=np.float32)
    b_v = np.asarray(b_v, dtype=np.float32)

    userT = np.ascontiguousarray(user.T)
    itemT = np.ascontiguousarray(item.T)
    userT8 = userT.astype(NP_FP8)
    itemT8 = itemT.astype(NP_FP8)
    UV8 = UV_adj.astype(NP_FP8)
    VU8 = np.ascontiguousarray(UV8.T)
    W_uT = np.ascontiguousarray(W_u.T)
    W_vT = np.ascontiguousarray(W_v.T)
    # [128, KH, H] fp8 weight layout for DoubleRow projections
    WfA_np = np.ascontiguousarray(
        W_vT.reshape(KH, 128, H).transpose(1, 0, 2).astype(NP_FP8))
    WfB_np = np.ascontiguousarray(
        W_uT.reshape(KH, 128, H).transpose(1, 0, 2).astype(NP_FP8))
    bfA_np = np.ascontiguousarray(b_v.reshape(KH, 128).T)
    bfB_np = np.ascontiguousarray(b_u.reshape(KH, 128).T)
    # bias for m-chunks 0,1 replicated along the free dim (DVE wide casts)
    bfA_rep_np = np.ascontiguousarray(
        np.broadcast_to(b_v.reshape(KH, 128).T[:, 0:2, None], (128, 2, H)))
    bfB_rep_np = np.ascontiguousarray(
        np.broadcast_to(b_u.reshape(KH, 128).T[:, 0:2, None], (128, 2, H)))
    brow_np = np.ascontiguousarray(
        np.broadcast_to((b_u + b_v)[None, None, :], (128, 2, H)))

    def tile_feat(f8):
        # [H, N] -> [NJ, H, 512] so each projection DMA is contiguous
        return np.ascontiguousarray(f8.reshape(H, NJ, 512).transpose(1, 0, 2))

    def tile_qt(qt):
        # [H, RB] -> [2, H, 512]
        return np.ascontiguousarray(qt.reshape(H, 2, 512).transpose(1, 0, 2))

    def tile_mask(m8):
        # [N, RB] -> [NRB, N, RBQ] so each mask DMA is contiguous
        return np.ascontiguousarray(m8.reshape(N, NRB, RBQ).transpose(1, 0, 2))

    in_maps = []
    for i in range(NCORES):
        r = i * RB
        sl = slice(r, r + RB)
        in_maps.append({
            # feature matrices with this core's rows rolled to the front
            "featA": tile_feat(np.roll(itemT8, -r, axis=1)),
            "featB": tile_feat(np.roll(userT8, -r, axis=1)),
            "qtA": tile_qt(userT[:, sl]),
            "qtB": tile_qt(itemT[:, sl]),
            "maskA": tile_mask(np.roll(VU8[:, sl], -r, axis=0)),
            "maskB": tile_mask(np.roll(UV8[:, sl], -r, axis=0)),
            "WfA": WfA_np,
            "WfB": WfB_np,
            "WqA": W_uT,
            "WqB": W_vT,
            "bfA": bfA_np,
            "bfB": bfB_np,
            "bfA_rep": bfA_rep_np,
            "bfB_rep": bfB_rep_np,
            "brow": brow_np,
        })

    if _NC_CACHE is None:
        _NC_CACHE = build_nc()
    res = run_bass_kernel_spmd(_NC_CACHE, in_maps, core_ids=list(range(NCORES)),
                               trace=TRACE)
    LAST_RESULT = res
    results = res.results
    learn_user = np.concatenate([results[i]["out"][:RB] for i in range(NCORES)], 0)
    learn_item = np.concatenate([results[i]["out"][RB:] for i in range(NCORES)], 0)
    return (learn_user, learn_item)


if __name__ == "__main__":
    nc = build_nc()
    print("built ok")


# revision 19
# speedup vs baseline: 1.2036x; 1.0011x over previous
"""Bipartite GNN attention kernel for Trainium2, SPMD across 8 NeuronCores.

Math (per reference):
  u = user @ W_u.T + b_u ; v = item @ W_v.T + b_v
  learn_user = softmax((u @ v.T) * UV_adj * scale, axis=1) @ v + u
  learn_item = softmax((v @ u.T) * VU_adj * scale, axis=1) @ u + v

Sharding: core i owns rows [i*1024, (i+1)*1024) of BOTH outputs; no
collectives (the contracted-side projection is replicated).

v6 design (fp8 DoubleRow, long streams):
- All big matmuls run in fp8e4 DoubleRow with 512-column outputs: long
  moving streams hide the (FWL-less) DoubleRow weight loads. 256-column
  variants measured issue/LDW-bound, not stream-bound.
- Feature matrices are projected twice: fT [h, N] (feature-major, biased,
  score lhsT) and vrow [N, h] (row-major, UNbiased, aggregation rhs).
  The missing bias in vrow cancels through softmax:
  P@(v + 1 b^T)/rsum = P@vrow/rsum + b^T, so b_feat is folded into the
  residual qrow instead. No per-block PE transposes anywhere.
- Per-core inputs are column-ROLLED so this core's rows are columns
  [0:RB) of both feature matrices; the score rhs (qTb) is then just
  fT_other[:, :, 0:RB] - no separate query projection.
- exp uses bias -ln(32): softmax is shift-invariant, masked entries
  become exactly 1/32 (fp8-exact), max value ~5 stays far below fp8e4
  max 240. Both exp halves of an item-chunk pair run as ONE Act op into
  a [128, 2, 512] fp8 pair tile (SBUF, so pairing is free).
- Denominator: ones^T @ pbf_pair, ONE DoubleRow matmul per pair with the
  2-column ones as the stationary side, accumulating a [1, 512] PSUM row;
  the epilogue transposes it to [128, 4] with four 1-partition matmuls.
- Aggregation of pair bp-2 is emitted after scores of pair bp so the PE
  never waits on the DVE-mult + Act-exp chain.
- All DMAs are fully contiguous: host pre-tiles masks per row-block and
  features per 512-column block.
- PSUM pools are phase-scoped: projections use a 3-deep ring of 2-bank
  tiles (paired casts amortize DVE/Act per-op overhead); attention uses
  agg[4] + scores[3] + rsum-row[1], with the epilogue transpose borrowing
  a scores slot.
"""

import sys

sys.path.insert(0, "/opt/trn_rl_repo")

import ml_dtypes
import numpy as np

import concourse.bacc as bacc
import concourse.bass as bass
import concourse.mybir as mybir
import concourse.tile as tile
from concourse.bass_utils import run_bass_kernel_spmd

N = 8192          # users == items
H = 512           # hidden
NCORES = 8
RB = N // NCORES  # 1024 rows per core per direction
KH = H // 128     # 4 h-chunks
NB = N // 128     # 64 column chunks
NBP = NB // 2     # 32 column-pair chunks (DoubleRow)
RBQ = 512         # users per attention row-block
NRB = RB // RBQ   # 2 r-blocks of 512
NJ = N // 512     # 16 512-col blocks for projection streaming
AGG_DEPTH = 4     # aggregation trails scores by this many pairs
SCALE = float(1.0 / np.sqrt(np.float32(H)))
NLN32 = float(-np.log(32.0))

F32 = mybir.dt.float32
F32R = mybir.dt.float32r
BF16 = mybir.dt.bfloat16
FP8 = mybir.dt.float8e4
NP_FP8 = ml_dtypes.float8_e4m3
DR = mybir.MatmulPerfMode.DoubleRow


def _r(ap):
    return ap.bitcast(F32R)


def build_nc():
    nc = bacc.Bacc("TRN2", target_bir_lowering=False, debug=False)

    featA = nc.declare_dram_parameter("featA", [NJ, H, 512], FP8, isOutput=False)
    featB = nc.declare_dram_parameter("featB", [NJ, H, 512], FP8, isOutput=False)
    qtA = nc.declare_dram_parameter("qtA", [2, H, 512], F32, isOutput=False)
    qtB = nc.declare_dram_parameter("qtB", [2, H, 512], F32, isOutput=False)
    maskA = nc.declare_dram_parameter("maskA", [NRB, N, RBQ], FP8, isOutput=False)
    maskB = nc.declare_dram_parameter("maskB", [NRB, N, RBQ], FP8, isOutput=False)
    WfA = nc.declare_dram_parameter("WfA", [128, KH, H], FP8, isOutput=False)
    WfB = nc.declare_dram_parameter("WfB", [128, KH, H], FP8, isOutput=False)
    WqA = nc.declare_dram_parameter("WqA", [H, H], F32, isOutput=False)
    WqB = nc.declare_dram_parameter("WqB", [H, H], F32, isOutput=False)
    bfA = nc.declare_dram_parameter("bfA", [128, KH], F32, isOutput=False)
    bfB = nc.declare_dram_parameter("bfB", [128, KH], F32, isOutput=False)
    brow = nc.declare_dram_parameter("brow", [128, 2, H], F32, isOutput=False)
    out = nc.declare_dram_parameter("out", [2 * RB, H], F32, isOutput=True)

    with tile.TileContext(nc) as tc:
        with (
            tc.tile_pool(name="bigA", bufs=1) as bigA,
            tc.tile_pool(name="bigB", bufs=1) as bigB,
            tc.tile_pool(name="wts", bufs=1) as wts,
            tc.tile_pool(name="stream", bufs=3) as stream,
            tc.tile_pool(name="qstream", bufs=1) as qstream,
            tc.tile_pool(name="mask", bufs=3) as maskp,
            tc.tile_pool(name="pf", bufs=3) as pfp,
            tc.tile_pool(name="pb", bufs=6) as pbp,
            tc.tile_pool(name="outs", bufs=1) as outsp,
            tc.tile_pool(name="small", bufs=1) as small,
        ):
            ones2 = small.tile([128, 2, 16], FP8, tag="ones")
            nc.vector.memset(ones2[:], 1.0)
            onesf = small.tile([1, 1], F32, tag="onesf")
            nc.vector.memset(onesf[:], 1.0)
            nbias = small.tile([128, 1], F32, tag="nbias")
            nc.vector.memset(nbias[:], NLN32)
            brow_sb = small.tile([128, 2, H], F32, tag="brow")
            nc.sync.dma_start(brow_sb[:], brow[:])
            bf_sb = {}
            for d, nar in (("A", bfA), ("B", bfB)):
                bf_sb[d] = small.tile([128, KH], F32, tag=f"bf{d}",
                                      name=f"bf{d}sb")
                nc.sync.dma_start(bf_sb[d][:], nar[:])

            # persistent per-direction tensors
            fT = {}
            vrow = {}
            qrow = {}
            for big_pool, d in ((bigA, "A"), (bigB, "B")):
                fT[d] = big_pool.tile([128, KH, N], FP8, tag=f"fT{d}",
                                      name=f"fT{d}")
                vrow[d] = big_pool.tile([128, NB, H], FP8, tag=f"vrow{d}",
                                        name=f"vrow{d}")
                qrow[d] = big_pool.tile([128, 2 * KH, H], BF16, tag=f"qrow{d}",
                                        name=f"qrow{d}")

            # ---------------- phase 0: projections ----------------
            def project(ps_w, d, feat_dram, qt_dram, wf_dram, wq_dram):
                wfp = wts.tile([128, KH, H], FP8, tag="wfp", name=f"wfp{d}")
                nc.sync.dma_start(wfp[:], wf_dram[:])
                wq = [wts.tile([128, H], F32R, tag=f"wq{k}", name=f"wq{d}{k}")
                      for k in range(KH)]
                for k in range(KH):
                    nc.sync.dma_start(
                        wq[k][:], wq_dram[k * 128:(k + 1) * 128, :].bitcast(F32R))

                # qrow: residual projection, f32r for accuracy; bias row
                # (b_q + b_feat) added from a replicated tile
                for half in range(2):
                    qt_in = [qstream.tile([128, 512], F32R, tag=f"qt{k}",
                                          name=f"qt{d}{half}{k}")
                             for k in range(KH)]
                    for k in range(KH):
                        nc.sync.dma_start(
                            qt_in[k][:],
                            qt_dram[half, k * 128:(k + 1) * 128, :].bitcast(F32R))
                    for cp in range(2):  # pairs of 128-row chunks
                        ps = ps_w.tile([128, 2, H], F32, tag="w")
                        for i in range(2):
                            c = cp * 2 + i
                            for k in range(KH):
                                nc.tensor.matmul(
                                    ps[:, i, :],
                                    qt_in[k][:, c * 128:(c + 1) * 128],
                                    wq[k][:],
                                    start=(k == 0), stop=(k == KH - 1))
                        nc.vector.tensor_tensor(
                            out=qrow[d][:, half * 4 + cp * 2:
                                        half * 4 + cp * 2 + 2, :],
                            in0=ps[:], in1=brow_sb[:],
                            op=mybir.AluOpType.add)

                # fT + vrow: fp8 DoubleRow projections with paired casts
                for j in range(NJ):
                    ft_in = stream.tile([128, KH, 512], FP8, tag="ft",
                                        name=f"ft{d}{j}")
                    for k in range(KH):
                        nc.sync.dma_start(
                            ft_in[:, k, :],
                            feat_dram[j, k * 128:(k + 1) * 128, :])
                    for mp in range(2):  # m-pairs
                        ps = ps_w.tile([128, 2, 512], F32, tag="w")
                        for i in range(2):
                            m = 2 * mp + i
                            for ko in range(2):
                                nc.tensor.matmul(
                                    ps[:, i, :],
                                    wfp[:, 2 * ko:2 * ko + 2,
                                        m * 128:(m + 1) * 128],
                                    ft_in[:, 2 * ko:2 * ko + 2, :],
                                    start=(ko == 0), stop=(ko == 1),
                                    perf_mode=DR)
                        for i in range(2):
                            m = 2 * mp + i
                            if mp == 0:
                                nc.vector.tensor_scalar(
                                    out=fT[d][:, m, j * 512:(j + 1) * 512],
                                    in0=ps[:, i, :],
                                    scalar1=bf_sb[d][:, m:m + 1], scalar2=None,
                                    op0=mybir.AluOpType.add)
                            else:
                                nc.scalar.add(
                                    fT[d][:, m, j * 512:(j + 1) * 512],
                                    ps[:, i, :], bf_sb[d][:, m:m + 1])
                    for sp in range(2):  # pairs of 128-item chunks
                        ps = ps_w.tile([128, 2, 512], F32, tag="w")
                        for i in range(2):
                            sub = sp * 2 + i
                            for ko in range(2):
                                nc.tensor.matmul(
                                    ps[:, i, :],
                                    ft_in[:, 2 * ko:2 * ko + 2,
                                          sub * 128:(sub + 1) * 128],
                                    wfp[:, 2 * ko:2 * ko + 2, :],
                                    start=(ko == 0), stop=(ko == 1),
                                    perf_mode=DR)
                        c = j * 4 + sp * 2
                        if sp == 0:
                            nc.vector.tensor_copy(vrow[d][:, c:c + 2, :], ps[:])
                        else:
                            nc.scalar.copy(vrow[d][:, c:c + 2, :], ps[:])

            with tc.tile_pool(name="ps_w", bufs=3, space="PSUM") as ps_w:
                project(ps_w, "A", featA, qtA, WfA, WqA)
                project(ps_w, "B", featB, qtB, WfB, WqB)

            # ---------------- attention ----------------
            def emit_agg(myvrow, agg, rs_row, pbf2, bp):
                for rs in range(4):
                    nc.tensor.matmul(
                        agg[:, rs, :], pbf2[:, :, rs * 128:(rs + 1) * 128],
                        myvrow[:, 2 * bp:2 * bp + 2, :],
                        start=(bp == 0), stop=(bp == NBP - 1), perf_mode=DR)
                # denominator: ones^T @ pbf2 -> [1, RBQ] row; ones is the
                # stationary operand so the weight load is only 2 columns
                nc.tensor.matmul(
                    rs_row[:], ones2[:, :, 0:1], pbf2[:],
                    start=(bp == 0), stop=(bp == NBP - 1), perf_mode=DR)

            def attention(ps_s, ps_agg, ps_rs, d, other, mask_dram, out_base):
                myfT = fT[d]
                myvrow = vrow[d]
                qTb = fT[other]
                epi = [None]  # deferred epilogue from the previous row-block

                def run_epilogue():
                    if epi[0] is not None:
                        epi[0]()
                        epi[0] = None

                for rb in range(NRB):
                    agg = ps_agg.tile([128, KH, 512], F32, tag="agg",
                                      name=f"agg{d}{rb}")
                    rs_row = ps_rs.tile([1, RBQ], F32, tag="rs",
                                        name=f"rs{d}{rb}")
                    pend = []  # pbf pairs waiting for aggregation
                    for bp in range(NBP):
                        if bp == 1:
                            # previous block's epilogue runs here, hidden
                            # under this block's score stream
                            run_epilogue()
                        sps = []
                        for t in range(2):
                            b = 2 * bp + t
                            sp = ps_s.tile([128, 512], F32, tag="s")
                            for ko in range(2):
                                nc.tensor.matmul(
                                    sp[:],
                                    myfT[:, 2 * ko:2 * ko + 2,
                                         b * 128:(b + 1) * 128],
                                    qTb[:, 2 * ko:2 * ko + 2,
                                        rb * RBQ:(rb + 1) * RBQ],
                                    start=(ko == 0), stop=(ko == 1),
                                    perf_mode=DR)
                            sps.append(sp)

                        # aggregate an OLD pair: its exp had AGG_DEPTH
                        # periods to finish, so the PE never waits
                        if len(pend) >= AGG_DEPTH:
                            emit_agg(myvrow, agg, rs_row, *pend.pop(0))
                        mt = maskp.tile([128, 2, RBQ], FP8, tag="mk")
                        for t in range(2):
                            nc.sync.dma_start(
                                mt[:, t, :],
                                mask_dram[rb, (2 * bp + t) * 128:
                                          (2 * bp + t + 1) * 128, :])
                        pbf2 = pbp.tile([128, 2, RBQ], FP8, tag="pbf")
                        p32 = pfp.tile([128, 2, RBQ], F32, tag="p32")
                        for t in range(2):
                            nc.vector.tensor_tensor(
                                out=p32[:, t, :], in0=sps[t][:],
                                in1=mt[:, t, :], op=mybir.AluOpType.mult)
                        # ONE exp for both halves (contiguous SBUF pair)
                        nc.scalar.activation(
                            pbf2[:], p32[:],
                            mybir.ActivationFunctionType.Exp,
                            bias=nbias[:], scale=SCALE)
                        pend.append((pbf2, bp))
                    for p in pend:
                        emit_agg(myvrow, agg, rs_row, *p)

                    def epilogue(d=d, rb=rb, agg=agg, rs_row=rs_row):
                        # transpose [1, RBQ] denominators to [128, 4] via
                        # 1-partition matmuls, then out = agg/rsum + qrow
                        rs_sb = small.tile([1, RBQ], F32, tag="rs_sb")
                        nc.vector.tensor_copy(rs_sb[:], rs_row[:])
                        rsT = ps_s.tile([128, 4], F32, tag="s",
                                        name=f"rsT{d}{rb}")
                        for rs in range(4):
                            nc.tensor.matmul(
                                rsT[:, rs:rs + 1],
                                rs_sb[0:1, rs * 128:(rs + 1) * 128], onesf[:],
                                start=(rs == 0), stop=(rs == 3))
                        recip = small.tile([128, 4], F32, tag="recip")
                        nc.vector.reciprocal(recip[:], rsT[:])
                        for rs in range(4):
                            o_sb = outsp.tile([128, H], F32, tag=f"o{rs}",
                                              name=f"o{d}{rb}_{rs}")
                            if rs % 2 == 0:
                                nc.vector.tensor_scalar(
                                    out=o_sb[:], in0=agg[:, rs, :],
                                    scalar1=recip[:, rs:rs + 1], scalar2=None,
                                    op0=mybir.AluOpType.mult)
                                nc.vector.tensor_tensor(
                                    out=o_sb[:], in0=o_sb[:],
                                    in1=qrow[d][:, rb * 4 + rs, :],
                                    op=mybir.AluOpType.add)
                            else:
                                nc.scalar.mul(o_sb[:], agg[:, rs, :],
                                              recip[:, rs:rs + 1])
                                nc.gpsimd.tensor_tensor(
                                    out=o_sb[:], in0=o_sb[:],
                                    in1=qrow[d][:, rb * 4 + rs, :],
                                    op=mybir.AluOpType.add)
                            row0 = out_base + rb * RBQ + rs * 128
                            nc.sync.dma_start(out[row0:row0 + 128, :], o_sb[:])

                    epi[0] = epilogue
                run_epilogue()

            with (
                tc.tile_pool(name="ps_s", bufs=3, space="PSUM") as ps_s,
                tc.tile_pool(name="ps_agg", bufs=1, space="PSUM") as ps_agg,
                tc.tile_pool(name="ps_rs", bufs=1, space="PSUM") as ps_rs,
            ):
                attention(ps_s, ps_agg, ps_rs, "A", "B", maskA, 0)
                attention(ps_s, ps_agg, ps_rs, "B", "A", maskB, RB)

    nc.compile()
    return nc


_NC_CACHE = None
TRACE = False
LAST_RESULT = None


def kernel(user, item, UV_adj, VU_adj, W_u, b_u, W_v, b_v):
    global _NC_CACHE, LAST_RESULT
    user = np.asarray(user, dtype=np.float32)
    item = np.asarray(item, dtype=np.float32)
    UV_adj = np.asarray(UV_adj, dtype=np.float32)
    VU_adj = np.asarray(VU_adj, dtype=np.float32)
    W_u = np.asarray(W_u, dtype=np.float32)
    W_v = np.asarray(W_v, dtype=np.float32)
    b_u = np.asarray(b_u, dtype=np.float32)
    b_v = np.asarray(b_v, dtype=np.float32)

    userT = np.ascontiguousarray(user.T)
    itemT = np.ascontiguousarray(item.T)
    userT8 = userT.astype(NP_FP8)
    itemT8 = itemT.astype(NP_FP8)
    UV8 = UV_adj.astype(NP_FP8)
    VU8 = np.ascontiguousarray(UV8.T)
    W_uT = np.ascontiguousarray(W_u.T)
    W_vT = np.ascontiguousarray(W_v.T)
    # [128, KH, H] fp8 weight layout for DoubleRow projections
    WfA_np = np.ascontiguousarray(
        W_vT.reshape(KH, 128, H).transpose(1, 0, 2).astype(NP_FP8))
    WfB_np = np.ascontiguousarray(
        W_uT.reshape(KH, 128, H).transpose(1, 0, 2).astype(NP_FP8))
    bfA_np = np.ascontiguousarray(b_v.reshape(KH, 128).T)
    bfB_np = np.ascontiguousarray(b_u.reshape(KH, 128).T)
    brow_np = np.ascontiguousarray(
        np.broadcast_to((b_u + b_v)[None, None, :], (128, 2, H)))

    def tile_feat(f8):
        # [H, N] -> [NJ, H, 512] so each projection DMA is contiguous
        return np.ascontiguousarray(f8.reshape(H, NJ, 512).transpose(1, 0, 2))

    def tile_qt(qt):
        # [H, RB] -> [2, H, 512]
        return np.ascontiguousarray(qt.reshape(H, 2, 512).transpose(1, 0, 2))

    def tile_mask(m8):
        # [N, RB] -> [NRB, N, RBQ] so each mask DMA is contiguous
        return np.ascontiguousarray(m8.reshape(N, NRB, RBQ).transpose(1, 0, 2))

    in_maps = []
    for i in range(NCORES):
        r = i * RB
        sl = slice(r, r + RB)
        in_maps.append({
            # feature matrices with this core's rows rolled to the front
            "featA": tile_feat(np.roll(itemT8, -r, axis=1)),
            "featB": tile_feat(np.roll(userT8, -r, axis=1)),
            "qtA": tile_qt(userT[:, sl]),
            "qtB": tile_qt(itemT[:, sl]),
            "maskA": tile_mask(np.roll(VU8[:, sl], -r, axis=0)),
            "maskB": tile_mask(np.roll(UV8[:, sl], -r, axis=0)),
            "WfA": WfA_np,
            "WfB": WfB_np,
            "WqA": W_uT,
            "WqB": W_vT,
            "bfA": bfA_np,
            "bfB": bfB_np,
            "brow": brow_np,
        })

    if _NC_CACHE is None:
        _NC_CACHE = build_nc()
    res = run_bass_kernel_spmd(_NC_CACHE, in_maps, core_ids=list(range(NCORES)),
                               trace=TRACE)
    LAST_RESULT = res
    results = res.results
    learn_user = np.concatenate([results[i]["out"][:RB] for i in range(NCORES)], 0)
    learn_item = np.concatenate([results[i]["out"][RB:] for i in range(NCORES)], 0)
    return (learn_user, learn_item)


if __name__ == "__main__":
    nc = build_nc()
    print("built ok")
